# revision 11
# baseline (speedup 1.0000x reference)
"""Trainium2 Bass kernel for nn_NodeEncoder (per-type Linear over interleaved node types).

Problem: x [800000, 128] f32, W [8, 256, 128], b [8, 256].
Node n has type k = n % 8; y[n] = (W[k] * mask_k) @ x[n] + b[k], y [800000, 256].

Strategy (8 cores, data-parallel over graphs, weights replicated):
  - Each core gets 100000 consecutive nodes (12500 graphs), padded to
    100352 = 49 super-tiles of 2048 nodes (256 graphs).
  - x is cast to fp16 (round-to-nearest; the PE multiplies fp16 at FP22 so
    ~2.4e-4 per-element rel err) and laid out on the host in transposed
    slice form: x_in[s, d, 128*j + n] = x[2048*s + 16*n + j, d].  Each
    slice j of a super-tile is 128 nodes, ALL of type j%8, with the
    contraction dim d already on partitions — so a contiguous 512 KiB DMA
    per super-tile feeds matmuls directly, no on-device transpose.
  - For types with dim < 128 the host writes 1.0 into x column `dim`
    (masked region), so rows 0..dim of a slice are [x.T; ones] and the
    bias rides as contraction row `dim` of the weight tile
    (y = [x,1] @ [W^T; b]).  For the two dim-128 types the (exact fp32)
    bias is added by GpSimd after eviction.
  - fp16 matmul accumulates in fp32 PSUM; pairs of slices share one PSUM
    bank [128, 512] and ScalarE/VectorE alternate evicting two slices per
    op into the fp32 out tile [128, 4096], which maps linearly to 2048
    output rows -> one contiguous 2 MiB DMA out.  All DMAs are fully
    contiguous.
W is pre-masked + pre-transposed on host (it is tiny: 1 MB).
"""

import os
import sys

import numpy as np

for _p in ("/root/.axon_site", "/root/.axon_site/_ro/trn_rl_repo", "/root/.axon_site/_ro/pypackages"):
    if os.path.isdir(_p) and _p not in sys.path:
        sys.path.append(_p)

import concourse.bass as bass
import concourse.mybir as mybir
import concourse.tile as tile
from concourse import bacc
from concourse.bass_utils import run_bass_kernel_spmd

N_TYPES = 8
MAX_DIM = 128
FEAT = 256
N_GRAPHS = 100000
NODE_DIMS = np.array([16, 32, 64, 128, 64, 32, 16, 128], dtype=np.int32)

N_CORES = 8
NODES_PER_CORE = N_GRAPHS * N_TYPES // N_CORES  # 100000
SUPER_NODES = 2048          # nodes per super-tile (256 graphs)
N_SUPER = 49                # super-tiles per core
PAD_NODES = SUPER_NODES * N_SUPER  # 100352
SLICES = SUPER_NODES // 128  # 16 slices of 128 nodes per super-tile

_F32 = mybir.dt.float32
_F16 = mybir.dt.float16
BIG_TYPES = [3, 7]          # types with dim == 128 (bias added on GpSimd)
_nc_cache = {}


def _build_nc():
    if "nc" in _nc_cache:
        return _nc_cache["nc"]
    nc = bacc.Bacc("TRN2", target_bir_lowering=False, debug=False)
    x = nc.dram_tensor("x", [N_SUPER, 128, SUPER_NODES], _F16, kind="ExternalInput").ap()
    wtb = nc.dram_tensor("wtb", [128, N_TYPES * FEAT], _F16, kind="ExternalInput").ap()
    bias_full = nc.dram_tensor("bias_full", [128, len(BIG_TYPES) * FEAT], _F32, kind="ExternalInput").ap()
    y = nc.dram_tensor("y", [N_SUPER, 128, SLICES * FEAT], _F32, kind="ExternalOutput").ap()

    with tile.TileContext(nc) as tc:
        with (
            tc.tile_pool(name="const", bufs=1) as const,
            tc.tile_pool(name="xin", bufs=4) as xin_pool,
            tc.tile_pool(name="outsb", bufs=4) as out_pool,
            tc.tile_pool(name="ps_o", bufs=6, space="PSUM") as ps_o,
        ):
            wtb_sb = const.tile([128, N_TYPES * FEAT], _F16)
            nc.sync.dma_start(wtb_sb[:], wtb[:])
            bias_sb = const.tile([128, len(BIG_TYPES) * FEAT], _F32)
            nc.sync.dma_start(bias_sb[:], bias_full[:])

            for s in range(N_SUPER):
                xs = xin_pool.tile([128, SUPER_NODES], _F16)
                nc.sync.dma_start(xs[:], x[s])
                out_sb = out_pool.tile([128, SLICES * FEAT], _F32)
                for jp in range(SLICES // 2):
                    po = ps_o.tile([128, 2 * FEAT], _F32)
                    for half in range(2):
                        j = 2 * jp + half
                        k = j % N_TYPES
                        dim = int(NODE_DIMS[k])
                        # rows 0..dim of slice j are [x.T; ones] (host layout),
                        # so the bias rides as contraction row `dim` of wtb.
                        kk = dim + 1 if dim < 128 else 128
                        nc.tensor.matmul(
                            po[:, half * FEAT:(half + 1) * FEAT],
                            xs[0:kk, j * 128:(j + 1) * 128],
                            wtb_sb[0:kk, k * FEAT:(k + 1) * FEAT],
                            start=True, stop=True,
                        )
                    osl = out_sb[:, 2 * jp * FEAT:(2 * jp + 2) * FEAT]
                    if jp % 2 == 0:
                        nc.scalar.copy(osl, po[:])
                    else:
                        nc.vector.tensor_copy(osl, po[:])
                    for half in range(2):
                        j = 2 * jp + half
                        k = j % N_TYPES
                        if k in BIG_TYPES:
                            t = BIG_TYPES.index(k)
                            nc.gpsimd.tensor_add(
                                out_sb[:, j * FEAT:(j + 1) * FEAT],
                                out_sb[:, j * FEAT:(j + 1) * FEAT],
                                bias_sb[:, t * FEAT:(t + 1) * FEAT],
                            )
                nc.scalar.dma_start(y[s], out_sb[:])

    nc.finalize()
    _nc_cache["nc"] = nc
    return nc


def _prep_weights(W, b):
    mask = (np.arange(MAX_DIM)[None, None, :] < NODE_DIMS[:, None, None])
    W_eff = np.where(mask, W, 0).astype(np.float32)  # [T, F, D]
    # wtb[d, k*256+f]: rows 0..dim_k-1 = W_eff[k].T; row dim_k = b[k] (small types)
    wtb = np.zeros((MAX_DIM, N_TYPES * FEAT), dtype=np.float32)
    for k in range(N_TYPES):
        dim = int(NODE_DIMS[k])
        wtb[:dim, k * FEAT:(k + 1) * FEAT] = W_eff[k, :, :dim].T
        if dim < MAX_DIM:
            wtb[dim, k * FEAT:(k + 1) * FEAT] = b[k]
    bias_full = np.ascontiguousarray(
        np.broadcast_to(
            np.concatenate([b[k] for k in BIG_TYPES]).astype(np.float32)[None, :],
            (128, len(BIG_TYPES) * FEAT),
        )
    )
    return wtb.astype(np.float16), bias_full


def _prep_x_shard(x, c):
    """fp16, ones-column injected, transposed slice layout [N_SUPER, 128, 2048]:
    out[s, d, 128*j + n] = xc[2048*s + 16*n + j, d]."""
    xc = np.zeros((PAD_NODES, MAX_DIM), dtype=np.float32)
    xc[:NODES_PER_CORE] = x[c * NODES_PER_CORE:(c + 1) * NODES_PER_CORE]
    for k in range(N_TYPES):
        dim = int(NODE_DIMS[k])
        if dim < MAX_DIM:
            xc[k::N_TYPES, dim] = 1.0  # ones-row for the folded bias
    xh = xc.astype(np.float16).reshape(N_SUPER, 128, SLICES, MAX_DIM)  # [s, n, j, d]
    return np.ascontiguousarray(xh.transpose(0, 3, 2, 1)).reshape(N_SUPER, 128, SUPER_NODES)


def run(x, W, b, trace=False):
    nc = _build_nc()
    wtb, bias_full = _prep_weights(W, b)
    in_maps = []
    for c in range(N_CORES):
        in_maps.append({
            "x": _prep_x_shard(x, c),
            "wtb": wtb,
            "bias_full": bias_full,
        })
    res = run_bass_kernel_spmd(nc, in_maps, list(range(N_CORES)), trace=trace)
    y = np.empty((N_GRAPHS * N_TYPES, FEAT), dtype=np.float32)
    for c in range(N_CORES):
        yc = np.asarray(res.results[c]["y"]).reshape(PAD_NODES, FEAT)
        y[c * NODES_PER_CORE:(c + 1) * NODES_PER_CORE] = yc[:NODES_PER_CORE]
    return y, res


def kernel(**inputs):
    y, _ = run(inputs["x"], inputs["W"], inputs["b"])
    return y


if __name__ == "__main__":
    rng = np.random.default_rng(0)
    x = rng.standard_normal((N_GRAPHS * N_TYPES, MAX_DIM), dtype=np.float32)
    W = (rng.standard_normal((N_TYPES, FEAT, MAX_DIM), dtype=np.float32) * 0.05)
    b = (rng.standard_normal((N_TYPES, FEAT), dtype=np.float32) * 0.05)
    y, res = run(x, W, b)
    mask = (np.arange(MAX_DIM)[None, None, :] < NODE_DIMS[:, None, None])
    W_eff = np.where(mask, W, 0).astype(np.float32)
    idx = rng.integers(0, N_GRAPHS * N_TYPES, 256)
    exp = np.stack([W_eff[n % 8] @ x[n] + b[n % 8] for n in idx])
    act = y[idx]
    err = np.abs(act - exp).max() / (np.abs(exp).max() + 1e-30)
    print("spot-check rel err:", err)


# revision 14
# speedup vs baseline: 1.0573x; 1.0573x over previous
"""Trainium2 Bass kernel for nn_NodeEncoder (per-type Linear over interleaved node types).

Problem: x [800000, 128] f32, W [8, 256, 128], b [8, 256].
Node n has type k = n % 8; y[n] = (W[k] * mask_k) @ x[n] + b[k], y [800000, 256].

Strategy (8 cores, data-parallel over graphs, weights replicated):
  - Each core gets 100000 consecutive nodes (12500 graphs), padded to
    100352 = 49 super-tiles of 2048 nodes (256 graphs).
  - x is cast to fp16 (round-to-nearest; the PE multiplies fp16 at FP22 so
    ~2.4e-4 per-element rel err) and laid out on the host in transposed
    slice form: x_in[s, d, 128*j + n] = x[2048*s + 16*n + j, d].  Each
    slice j of a super-tile is 128 nodes, ALL of type j%8, with the
    contraction dim d already on partitions — so a contiguous 512 KiB DMA
    per super-tile feeds matmuls directly, no on-device transpose.
  - For types with dim < 128 the host writes 1.0 into x column `dim`
    (masked region), so rows 0..dim of a slice are [x.T; ones] and the
    bias rides as contraction row `dim` of the weight tile
    (y = [x,1] @ [W^T; b]).  For the two dim-128 types the (exact fp32)
    bias is added by GpSimd after eviction.
  - fp16 matmul accumulates in fp32 PSUM; pairs of slices share one PSUM
    bank [128, 512] and ScalarE/VectorE alternate evicting two slices per
    op into the fp32 out tile [128, 4096], which maps linearly to 2048
    output rows -> one contiguous 2 MiB DMA out.  All DMAs are fully
    contiguous.
W is pre-masked + pre-transposed on host (it is tiny: 1 MB).
"""

import os
import sys

import numpy as np

for _p in ("/root/.axon_site", "/root/.axon_site/_ro/trn_rl_repo", "/root/.axon_site/_ro/pypackages"):
    if os.path.isdir(_p) and _p not in sys.path:
        sys.path.append(_p)

import concourse.bass as bass
import concourse.mybir as mybir
import concourse.tile as tile
from concourse import bacc
from concourse.bass_utils import run_bass_kernel_spmd

N_TYPES = 8
MAX_DIM = 128
FEAT = 256
N_GRAPHS = 100000
NODE_DIMS = np.array([16, 32, 64, 128, 64, 32, 16, 128], dtype=np.int32)

N_CORES = 8
NODES_PER_CORE = N_GRAPHS * N_TYPES // N_CORES  # 100000
SUPER_NODES = 2048          # nodes per super-tile (256 graphs)
N_SUPER = 49                # super-tiles per core
PAD_NODES = SUPER_NODES * N_SUPER  # 100352
SLICES = SUPER_NODES // 128  # 16 slices of 128 nodes per super-tile

_F32 = mybir.dt.float32
_F16 = mybir.dt.float16
BIG_TYPES = [3, 7]          # types with dim == 128 (bias via K=1 accumulate matmul)
OUT_F16 = True              # store y as fp16 (halves write traffic; host upcasts)
_nc_cache = {}


def _build_nc():
    if "nc" in _nc_cache:
        return _nc_cache["nc"]
    out_dt = _F16 if OUT_F16 else _F32
    nc = bacc.Bacc("TRN2", target_bir_lowering=False, debug=False)
    x = nc.dram_tensor("x", [N_SUPER, 128, SUPER_NODES], _F16, kind="ExternalInput").ap()
    wtb = nc.dram_tensor("wtb", [128, N_TYPES * FEAT], _F16, kind="ExternalInput").ap()
    bvec = nc.dram_tensor("bvec", [1, len(BIG_TYPES) * FEAT], _F16, kind="ExternalInput").ap()
    ones_in = nc.dram_tensor("ones_in", [1, 128], _F16, kind="ExternalInput").ap()
    y = nc.dram_tensor("y", [N_SUPER, 128, SLICES * FEAT], out_dt, kind="ExternalOutput").ap()

    with tile.TileContext(nc) as tc:
        with (
            tc.tile_pool(name="const", bufs=1) as const,
            tc.tile_pool(name="xin", bufs=4) as xin_pool,
            tc.tile_pool(name="outsb", bufs=4) as out_pool,
            tc.tile_pool(name="ps_o", bufs=6, space="PSUM") as ps_o,
        ):
            wtb_sb = const.tile([128, N_TYPES * FEAT], _F16)
            nc.sync.dma_start(wtb_sb[:], wtb[:])
            b_sb = const.tile([1, len(BIG_TYPES) * FEAT], _F16)
            nc.sync.dma_start(b_sb[:], bvec[:])
            ones = const.tile([1, 128], _F16)
            nc.sync.dma_start(ones[:], ones_in[:])

            for s in range(N_SUPER):
                xs = xin_pool.tile([128, SUPER_NODES], _F16)
                nc.sync.dma_start(xs[:], x[s])
                out_sb = out_pool.tile([128, SLICES * FEAT], out_dt)
                for jp in range(SLICES // 2):
                    po = ps_o.tile([128, 2 * FEAT], _F32)
                    for half in range(2):
                        j = 2 * jp + half
                        k = j % N_TYPES
                        dim = int(NODE_DIMS[k])
                        # rows 0..dim of slice j are [x.T; ones] (host layout),
                        # so the bias rides as contraction row `dim` of wtb.
                        # dim-128 types have no spare row: bias comes from a
                        # K=1 matmul (ones.T @ b) accumulated in PSUM instead.
                        kk = dim + 1 if dim < 128 else 128
                        ph = po[:, half * FEAT:(half + 1) * FEAT]
                        if dim == 128:
                            t = BIG_TYPES.index(k)
                            nc.tensor.matmul(
                                ph, ones[:], b_sb[:, t * FEAT:(t + 1) * FEAT],
                                start=True, stop=False,
                            )
                        nc.tensor.matmul(
                            ph,
                            xs[0:kk, j * 128:(j + 1) * 128],
                            wtb_sb[0:kk, k * FEAT:(k + 1) * FEAT],
                            start=(dim < 128), stop=True,
                        )
                    osl = out_sb[:, 2 * jp * FEAT:(2 * jp + 2) * FEAT]
                    if jp % 2 == 0:
                        nc.scalar.copy(osl, po[:])
                    else:
                        nc.vector.tensor_copy(osl, po[:])
                nc.scalar.dma_start(y[s], out_sb[:])

    nc.finalize()
    _nc_cache["nc"] = nc
    return nc


def _prep_weights(W, b):
    mask = (np.arange(MAX_DIM)[None, None, :] < NODE_DIMS[:, None, None])
    W_eff = np.where(mask, W, 0).astype(np.float32)  # [T, F, D]
    # wtb[d, k*256+f]: rows 0..dim_k-1 = W_eff[k].T; row dim_k = b[k] (small types)
    wtb = np.zeros((MAX_DIM, N_TYPES * FEAT), dtype=np.float32)
    for k in range(N_TYPES):
        dim = int(NODE_DIMS[k])
        wtb[:dim, k * FEAT:(k + 1) * FEAT] = W_eff[k, :, :dim].T
        if dim < MAX_DIM:
            wtb[dim, k * FEAT:(k + 1) * FEAT] = b[k]
    bvec = np.concatenate([b[k] for k in BIG_TYPES]).astype(np.float16)[None, :]
    return wtb.astype(np.float16), np.ascontiguousarray(bvec)


def _prep_x_shard(x, c):
    """fp16, ones-column injected, transposed slice layout [N_SUPER, 128, 2048]:
    out[s, d, 128*j + n] = xc[2048*s + 16*n + j, d]."""
    xc = np.zeros((PAD_NODES, MAX_DIM), dtype=np.float32)
    xc[:NODES_PER_CORE] = x[c * NODES_PER_CORE:(c + 1) * NODES_PER_CORE]
    for k in range(N_TYPES):
        dim = int(NODE_DIMS[k])
        if dim < MAX_DIM:
            xc[k::N_TYPES, dim] = 1.0  # ones-row for the folded bias
    xh = xc.astype(np.float16).reshape(N_SUPER, 128, SLICES, MAX_DIM)  # [s, n, j, d]
    return np.ascontiguousarray(xh.transpose(0, 3, 2, 1)).reshape(N_SUPER, 128, SUPER_NODES)


def run(x, W, b, trace=False):
    nc = _build_nc()
    wtb, bvec = _prep_weights(W, b)
    ones = np.ones((1, 128), dtype=np.float16)
    in_maps = []
    for c in range(N_CORES):
        in_maps.append({
            "x": _prep_x_shard(x, c),
            "wtb": wtb,
            "bvec": bvec,
            "ones_in": ones,
        })
    res = run_bass_kernel_spmd(nc, in_maps, list(range(N_CORES)), trace=trace)
    y = np.empty((N_GRAPHS * N_TYPES, FEAT), dtype=np.float32)
    for c in range(N_CORES):
        yc = np.asarray(res.results[c]["y"]).reshape(PAD_NODES, FEAT)
        y[c * NODES_PER_CORE:(c + 1) * NODES_PER_CORE] = yc[:NODES_PER_CORE].astype(np.float32)
    return y, res


def kernel(**inputs):
    y, _ = run(inputs["x"], inputs["W"], inputs["b"])
    return y


if __name__ == "__main__":
    rng = np.random.default_rng(0)
    x = rng.standard_normal((N_GRAPHS * N_TYPES, MAX_DIM), dtype=np.float32)
    W = (rng.standard_normal((N_TYPES, FEAT, MAX_DIM), dtype=np.float32) * 0.05)
    b = (rng.standard_normal((N_TYPES, FEAT), dtype=np.float32) * 0.05)
    y, res = run(x, W, b)
    mask = (np.arange(MAX_DIM)[None, None, :] < NODE_DIMS[:, None, None])
    W_eff = np.where(mask, W, 0).astype(np.float32)
    idx = rng.integers(0, N_GRAPHS * N_TYPES, 256)
    exp = np.stack([W_eff[n % 8] @ x[n] + b[n % 8] for n in idx])
    act = y[idx]
    err = np.abs(act - exp).max() / (np.abs(exp).max() + 1e-30)
    print("spot-check rel err:", err)


# revision 18
# speedup vs baseline: 1.2538x; 1.1858x over previous
"""Trainium2 Bass kernel for nn_NodeEncoder (per-type Linear over interleaved node types).

Problem: x [800000, 128] f32, W [8, 256, 128], b [8, 256].
Node n has type k = n % 8; y[n] = (W[k] * mask_k) @ x[n] + b[k], y [800000, 256].

Strategy (8 cores, data-parallel over graphs, weights replicated):
  - Each core gets 100000 consecutive nodes (12500 graphs), padded to
    100352 = 49 super-tiles of 2048 nodes (256 graphs).
  - x is cast to fp16 (round-to-nearest; the PE multiplies fp16 at FP22 so
    ~2.4e-4 per-element rel err) and laid out on the host in transposed
    slice form: x_in[s, d, 128*j + n] = x[2048*s + 16*n + j, d].  Each
    slice j of a super-tile is 128 nodes, ALL of type j%8, with the
    contraction dim d already on partitions — so a contiguous 512 KiB DMA
    per super-tile feeds matmuls directly, no on-device transpose.
  - For types with dim < 128 the host writes 1.0 into x column `dim`
    (masked region), so rows 0..dim of a slice are [x.T; ones] and the
    bias rides as contraction row `dim` of the weight tile
    (y = [x,1] @ [W^T; b]).  For the two dim-128 types the (exact fp32)
    bias is added by GpSimd after eviction.
  - fp16 matmul accumulates in fp32 PSUM; pairs of slices share one PSUM
    bank [128, 512] and ScalarE/VectorE alternate evicting two slices per
    op into the fp32 out tile [128, 4096], which maps linearly to 2048
    output rows -> one contiguous 2 MiB DMA out.  All DMAs are fully
    contiguous.
W is pre-masked + pre-transposed on host (it is tiny: 1 MB).
"""

import os
import sys

import numpy as np

for _p in ("/root/.axon_site", "/root/.axon_site/_ro/trn_rl_repo", "/root/.axon_site/_ro/pypackages"):
    if os.path.isdir(_p) and _p not in sys.path:
        sys.path.append(_p)

import concourse.bass as bass
import concourse.mybir as mybir
import concourse.tile as tile
from concourse import bacc
from concourse.bass_utils import run_bass_kernel_spmd

N_TYPES = 8
MAX_DIM = 128
FEAT = 256
N_GRAPHS = 100000
NODE_DIMS = np.array([16, 32, 64, 128, 64, 32, 16, 128], dtype=np.int32)

N_CORES = 8
NODES_PER_CORE = N_GRAPHS * N_TYPES // N_CORES  # 100000
SUPER_NODES = 2048          # nodes per super-tile (256 graphs)
N_SUPER = 49                # super-tiles per core
PAD_NODES = SUPER_NODES * N_SUPER  # 100352
SLICES = SUPER_NODES // 128  # 16 slices of 128 nodes per super-tile

_F32 = mybir.dt.float32
_F16 = mybir.dt.float16
BIG_TYPES = [3, 7]          # types with dim == 128 (bias via K=1 accumulate matmul)
OUT_F16 = True              # store y as fp16 (halves write traffic; host upcasts)
_nc_cache = {}


def _build_nc():
    if "nc" in _nc_cache:
        return _nc_cache["nc"]
    out_dt = _F16 if OUT_F16 else _F32
    nc = bacc.Bacc("TRN2", target_bir_lowering=False, debug=False)
    x = nc.dram_tensor("x", [N_SUPER, 128, SUPER_NODES], _F16, kind="ExternalInput").ap()
    wtb = nc.dram_tensor("wtb", [128, N_TYPES * FEAT], _F16, kind="ExternalInput").ap()
    # per-pair bias tiles for the slice pairs containing a dim-128 type:
    # [:, 0:512] for pair (2,3) = [0 | b3], [:, 512:1024] for (6,7) = [0 | b7]
    bias_pair = nc.dram_tensor("bias_pair", [128, 2 * 2 * FEAT], _F32, kind="ExternalInput").ap()
    y = nc.dram_tensor("y", [N_SUPER, 128, SLICES * FEAT], out_dt, kind="ExternalOutput").ap()

    with tile.TileContext(nc) as tc:
        with (
            tc.tile_pool(name="const", bufs=1) as const,
            tc.tile_pool(name="xin", bufs=4) as xin_pool,
            tc.tile_pool(name="outsb", bufs=4) as out_pool,
            tc.tile_pool(name="ps_o", bufs=6, space="PSUM") as ps_o,
        ):
            wtb_sb = const.tile([128, N_TYPES * FEAT], _F16)
            nc.sync.dma_start(wtb_sb[:], wtb[:])
            bp_sb = const.tile([128, 2 * 2 * FEAT], _F32)
            nc.sync.dma_start(bp_sb[:], bias_pair[:])

            for s in range(N_SUPER):
                xs = xin_pool.tile([128, SUPER_NODES], _F16)
                nc.sync.dma_start(xs[:], x[s])
                out_sb = out_pool.tile([128, SLICES * FEAT], out_dt)
                for jp in range(SLICES // 2):
                    po = ps_o.tile([128, 2 * FEAT], _F32)
                    for half in range(2):
                        j = 2 * jp + half
                        k = j % N_TYPES
                        dim = int(NODE_DIMS[k])
                        # rows 0..dim of slice j are [x.T; ones] (host layout),
                        # so the bias rides as contraction row `dim` of wtb.
                        kk = dim + 1 if dim < 128 else 128
                        nc.tensor.matmul(
                            po[:, half * FEAT:(half + 1) * FEAT],
                            xs[0:kk, j * 128:(j + 1) * 128],
                            wtb_sb[0:kk, k * FEAT:(k + 1) * FEAT],
                            start=True, stop=True,
                        )
                    osl = out_sb[:, 2 * jp * FEAT:(2 * jp + 2) * FEAT]
                    if jp % 4 == 1:   # pair (2,3): add [0 | b3] during eviction
                        nc.vector.tensor_add(osl, po[:], bp_sb[:, 0:2 * FEAT])
                    elif jp % 4 == 3:  # pair (6,7): add [0 | b7]
                        nc.vector.tensor_add(osl, po[:], bp_sb[:, 2 * FEAT:4 * FEAT])
                    else:              # unbiased pairs evict on ScalarE
                        nc.scalar.copy(osl, po[:])
                nc.scalar.dma_start(y[s], out_sb[:])

    nc.finalize()
    _nc_cache["nc"] = nc
    return nc


def _prep_weights(W, b):
    mask = (np.arange(MAX_DIM)[None, None, :] < NODE_DIMS[:, None, None])
    W_eff = np.where(mask, W, 0).astype(np.float32)  # [T, F, D]
    # wtb[d, k*256+f]: rows 0..dim_k-1 = W_eff[k].T; row dim_k = b[k] (small types)
    wtb = np.zeros((MAX_DIM, N_TYPES * FEAT), dtype=np.float32)
    for k in range(N_TYPES):
        dim = int(NODE_DIMS[k])
        wtb[:dim, k * FEAT:(k + 1) * FEAT] = W_eff[k, :, :dim].T
        if dim < MAX_DIM:
            wtb[dim, k * FEAT:(k + 1) * FEAT] = b[k]
    # bias_pair [128, 1024] f32: [0 | b3] then [0 | b7], broadcast over partitions
    bp = np.zeros((2, 2 * FEAT), dtype=np.float32)
    for t, k in enumerate(BIG_TYPES):
        bp[t, FEAT:] = b[k]
    bias_pair = np.ascontiguousarray(
        np.broadcast_to(bp.reshape(1, 4 * FEAT), (128, 4 * FEAT))
    )
    return wtb.astype(np.float16), bias_pair


def _prep_x_shard(x, c):
    """fp16, ones-column injected, transposed slice layout [N_SUPER, 128, 2048]:
    out[s, d, 128*j + n] = xc[2048*s + 16*n + j, d]."""
    xc = np.zeros((PAD_NODES, MAX_DIM), dtype=np.float32)
    xc[:NODES_PER_CORE] = x[c * NODES_PER_CORE:(c + 1) * NODES_PER_CORE]
    for k in range(N_TYPES):
        dim = int(NODE_DIMS[k])
        if dim < MAX_DIM:
            xc[k::N_TYPES, dim] = 1.0  # ones-row for the folded bias
    xh = xc.astype(np.float16).reshape(N_SUPER, 128, SLICES, MAX_DIM)  # [s, n, j, d]
    return np.ascontiguousarray(xh.transpose(0, 3, 2, 1)).reshape(N_SUPER, 128, SUPER_NODES)


def run(x, W, b, trace=False):
    nc = _build_nc()
    wtb, bias_pair = _prep_weights(W, b)
    in_maps = []
    for c in range(N_CORES):
        in_maps.append({
            "x": _prep_x_shard(x, c),
            "wtb": wtb,
            "bias_pair": bias_pair,
        })
    res = run_bass_kernel_spmd(nc, in_maps, list(range(N_CORES)), trace=trace)
    y = np.empty((N_GRAPHS * N_TYPES, FEAT), dtype=np.float32)
    for c in range(N_CORES):
        yc = np.asarray(res.results[c]["y"]).reshape(PAD_NODES, FEAT)
        y[c * NODES_PER_CORE:(c + 1) * NODES_PER_CORE] = yc[:NODES_PER_CORE].astype(np.float32)
    return y, res


def kernel(**inputs):
    y, _ = run(inputs["x"], inputs["W"], inputs["b"])
    return y


if __name__ == "__main__":
    rng = np.random.default_rng(0)
    x = rng.standard_normal((N_GRAPHS * N_TYPES, MAX_DIM), dtype=np.float32)
    W = (rng.standard_normal((N_TYPES, FEAT, MAX_DIM), dtype=np.float32) * 0.05)
    b = (rng.standard_normal((N_TYPES, FEAT), dtype=np.float32) * 0.05)
    y, res = run(x, W, b)
    mask = (np.arange(MAX_DIM)[None, None, :] < NODE_DIMS[:, None, None])
    W_eff = np.where(mask, W, 0).astype(np.float32)
    idx = rng.integers(0, N_GRAPHS * N_TYPES, 256)
    exp = np.stack([W_eff[n % 8] @ x[n] + b[n % 8] for n in idx])
    act = y[idx]
    err = np.abs(act - exp).max() / (np.abs(exp).max() + 1e-30)
    print("spot-check rel err:", err)


# revision 24
# speedup vs baseline: 1.4538x; 1.1596x over previous
"""Trainium2 Bass kernel for nn_NodeEncoder (per-type Linear over interleaved node types).

Problem: x [800000, 128] f32, W [8, 256, 128], b [8, 256].
Node n has type k = n % 8; y[n] = (W[k] * mask_k) @ x[n] + b[k], y [800000, 256].

Strategy (8 cores, data-parallel over graphs, weights replicated):
  - Each core gets 100000 consecutive nodes (12500 graphs), padded to
    100352 = 49 super-tiles of 2048 nodes (256 graphs).
  - x is cast to fp16 (round-to-nearest; the PE multiplies fp16 at FP22 so
    ~2.4e-4 per-element rel err) and laid out on the host in transposed
    slice form: x_in[s, d, 128*j + n] = x[2048*s + 16*n + j, d].  Each
    slice j of a super-tile is 128 nodes, ALL of type j%8, with the
    contraction dim d already on partitions — so a contiguous 512 KiB DMA
    per super-tile feeds matmuls directly, no on-device transpose.
  - For types with dim < 128 the host writes 1.0 into x column `dim`
    (masked region), so rows 0..dim of a slice are [x.T; ones] and the
    bias rides as contraction row `dim` of the weight tile
    (y = [x,1] @ [W^T; b]).  For the two dim-128 types the (exact fp32)
    bias is added by GpSimd after eviction.
  - fp16 matmul accumulates in fp32 PSUM; pairs of slices share one PSUM
    bank [128, 512] and ScalarE/VectorE alternate evicting two slices per
    op into the fp32 out tile [128, 4096], which maps linearly to 2048
    output rows -> one contiguous 2 MiB DMA out.  All DMAs are fully
    contiguous.
W is pre-masked + pre-transposed on host (it is tiny: 1 MB).
"""

import os
import sys

import numpy as np

for _p in ("/root/.axon_site", "/root/.axon_site/_ro/trn_rl_repo", "/root/.axon_site/_ro/pypackages"):
    if os.path.isdir(_p) and _p not in sys.path:
        sys.path.append(_p)

import concourse.bass as bass
import concourse.mybir as mybir
import concourse.tile as tile
from concourse import bacc
from concourse.bass_utils import run_bass_kernel_spmd

N_TYPES = 8
MAX_DIM = 128
FEAT = 256
N_GRAPHS = 100000
NODE_DIMS = np.array([16, 32, 64, 128, 64, 32, 16, 128], dtype=np.int32)

N_CORES = 8
NODES_PER_CORE = N_GRAPHS * N_TYPES // N_CORES  # 100000
SUPER_NODES = 2048          # nodes per super-tile (256 graphs)
N_SUPER = 49                # super-tiles per core
PAD_NODES = SUPER_NODES * N_SUPER  # 100352
SLICES = SUPER_NODES // 128  # 16 slices of 128 nodes per super-tile
UNIT = 7                    # super-tiles per DMA unit (49 = 7 units of 7)
N_UNITS = N_SUPER // UNIT

_F32 = mybir.dt.float32
_F16 = mybir.dt.float16
BIG_TYPES = [3, 7]          # types with dim == 128 (bias via K=1 accumulate matmul)
OUT_F16 = True              # store y as fp16 (halves write traffic; host upcasts)
_nc_cache = {}


def _build_nc():
    if "nc" in _nc_cache:
        return _nc_cache["nc"]
    out_dt = _F16 if OUT_F16 else _F32
    nc = bacc.Bacc("TRN2", target_bir_lowering=False, debug=False)
    x = nc.dram_tensor("x", [N_UNITS, 128, UNIT * SUPER_NODES], _F16, kind="ExternalInput").ap()
    wtb = nc.dram_tensor("wtb", [128, N_TYPES * FEAT], _F16, kind="ExternalInput").ap()
    # per-pair bias tiles for the slice pairs containing a dim-128 type:
    # [:, 0:512] for pair (2,3) = [0 | b3], [:, 512:1024] for (6,7) = [0 | b7]
    bias_pair = nc.dram_tensor("bias_pair", [128, 2 * 2 * FEAT], _F32, kind="ExternalInput").ap()
    y = nc.dram_tensor("y", [N_UNITS, 128, UNIT * SLICES * FEAT], out_dt, kind="ExternalOutput").ap()

    with tile.TileContext(nc) as tc:
        with (
            tc.tile_pool(name="const", bufs=1) as const,
            tc.tile_pool(name="xin", bufs=2) as xin_pool,
            tc.tile_pool(name="outsb", bufs=2) as out_pool,
            tc.tile_pool(name="ps_o", bufs=6, space="PSUM") as ps_o,
        ):
            wtb_sb = const.tile([128, N_TYPES * FEAT], _F16)
            nc.sync.dma_start(wtb_sb[:], wtb[:])
            bp_sb = const.tile([128, 2 * 2 * FEAT], _F32)
            nc.sync.dma_start(bp_sb[:], bias_pair[:])

            for u in range(N_UNITS):
                xs = xin_pool.tile([128, UNIT * SUPER_NODES], _F16)
                nc.sync.dma_start(xs[:], x[u])
                out_sb = out_pool.tile([128, UNIT * SLICES * FEAT], out_dt)
                for st in range(UNIT):
                    xoff = st * SUPER_NODES
                    ooff = st * SLICES * FEAT
                    for jp in range(SLICES // 2):
                        po = ps_o.tile([128, 2 * FEAT], _F32)
                        for half in range(2):
                            j = 2 * jp + half
                            k = j % N_TYPES
                            dim = int(NODE_DIMS[k])
                            # rows 0..dim of slice j are [x.T; ones] (host
                            # layout), so the bias rides as contraction row
                            # `dim` of wtb.
                            kk = dim + 1 if dim < 128 else 128
                            nc.tensor.matmul(
                                po[:, half * FEAT:(half + 1) * FEAT],
                                xs[0:kk, xoff + j * 128:xoff + (j + 1) * 128],
                                wtb_sb[0:kk, k * FEAT:(k + 1) * FEAT],
                                start=True, stop=True,
                            )
                        osl = out_sb[:, ooff + 2 * jp * FEAT:ooff + (2 * jp + 2) * FEAT]
                        if jp % 4 == 1:   # pair (2,3): add [0 | b3] during eviction
                            nc.vector.tensor_add(osl, po[:], bp_sb[:, 0:2 * FEAT])
                        elif jp % 4 == 3:  # pair (6,7): add [0 | b7]
                            nc.vector.tensor_add(osl, po[:], bp_sb[:, 2 * FEAT:4 * FEAT])
                        else:              # unbiased pairs evict on ScalarE
                            nc.scalar.copy(osl, po[:])
                nc.scalar.dma_start(y[u], out_sb[:])

    nc.finalize()
    _nc_cache["nc"] = nc
    return nc


def _prep_weights(W, b):
    mask = (np.arange(MAX_DIM)[None, None, :] < NODE_DIMS[:, None, None])
    W_eff = np.where(mask, W, 0).astype(np.float32)  # [T, F, D]
    # wtb[d, k*256+f]: rows 0..dim_k-1 = W_eff[k].T; row dim_k = b[k] (small types)
    wtb = np.zeros((MAX_DIM, N_TYPES * FEAT), dtype=np.float32)
    for k in range(N_TYPES):
        dim = int(NODE_DIMS[k])
        wtb[:dim, k * FEAT:(k + 1) * FEAT] = W_eff[k, :, :dim].T
        if dim < MAX_DIM:
            wtb[dim, k * FEAT:(k + 1) * FEAT] = b[k]
    # bias_pair [128, 1024] f32: [0 | b3] then [0 | b7], broadcast over partitions
    bp = np.zeros((2, 2 * FEAT), dtype=np.float32)
    for t, k in enumerate(BIG_TYPES):
        bp[t, FEAT:] = b[k]
    bias_pair = np.ascontiguousarray(
        np.broadcast_to(bp.reshape(1, 4 * FEAT), (128, 4 * FEAT))
    )
    return wtb.astype(np.float16), bias_pair


def _prep_x_shard(x, c):
    """fp16, ones-column injected, transposed slice layout [N_SUPER, 128, 2048]:
    out[s, d, 128*j + n] = xc[2048*s + 16*n + j, d]."""
    xc = np.zeros((PAD_NODES, MAX_DIM), dtype=np.float32)
    xc[:NODES_PER_CORE] = x[c * NODES_PER_CORE:(c + 1) * NODES_PER_CORE]
    for k in range(N_TYPES):
        dim = int(NODE_DIMS[k])
        if dim < MAX_DIM:
            xc[k::N_TYPES, dim] = 1.0  # ones-row for the folded bias
    xh = xc.astype(np.float16).reshape(N_SUPER, 128, SLICES, MAX_DIM)  # [s, n, j, d]
    xt = np.ascontiguousarray(xh.transpose(0, 3, 2, 1)).reshape(N_SUPER, 128, SUPER_NODES)
    # group into DMA units of UNIT super-tiles: [u, p, st*2048 + c] = xt[7u+st, p, c]
    xu = xt.reshape(N_UNITS, UNIT, 128, SUPER_NODES).transpose(0, 2, 1, 3)
    return np.ascontiguousarray(xu).reshape(N_UNITS, 128, UNIT * SUPER_NODES)


def run(x, W, b, trace=False):
    nc = _build_nc()
    wtb, bias_pair = _prep_weights(W, b)
    in_maps = []
    for c in range(N_CORES):
        in_maps.append({
            "x": _prep_x_shard(x, c),
            "wtb": wtb,
            "bias_pair": bias_pair,
        })
    res = run_bass_kernel_spmd(nc, in_maps, list(range(N_CORES)), trace=trace)
    y = np.empty((N_GRAPHS * N_TYPES, FEAT), dtype=np.float32)
    for c in range(N_CORES):
        yu = np.asarray(res.results[c]["y"]).reshape(N_UNITS, 128, UNIT, SLICES * FEAT)
        yc = yu.transpose(0, 2, 1, 3).reshape(PAD_NODES, FEAT)
        y[c * NODES_PER_CORE:(c + 1) * NODES_PER_CORE] = yc[:NODES_PER_CORE].astype(np.float32)
    return y, res


def kernel(**inputs):
    y, _ = run(inputs["x"], inputs["W"], inputs["b"])
    return y


if __name__ == "__main__":
    rng = np.random.default_rng(0)
    x = rng.standard_normal((N_GRAPHS * N_TYPES, MAX_DIM), dtype=np.float32)
    W = (rng.standard_normal((N_TYPES, FEAT, MAX_DIM), dtype=np.float32) * 0.05)
    b = (rng.standard_normal((N_TYPES, FEAT), dtype=np.float32) * 0.05)
    y, res = run(x, W, b)
    mask = (np.arange(MAX_DIM)[None, None, :] < NODE_DIMS[:, None, None])
    W_eff = np.where(mask, W, 0).astype(np.float32)
    idx = rng.integers(0, N_GRAPHS * N_TYPES, 256)
    exp = np.stack([W_eff[n % 8] @ x[n] + b[n % 8] for n in idx])
    act = y[idx]
    err = np.abs(act - exp).max() / (np.abs(exp).max() + 1e-30)
    print("spot-check rel err:", err)


# revision 30
# speedup vs baseline: 1.5452x; 1.0629x over previous
"""Trainium2 Bass kernel for nn_NodeEncoder (per-type Linear over interleaved node types).

Problem: x [800000, 128] f32, W [8, 256, 128], b [8, 256].
Node n has type k = n % 8; y[n] = (W[k] * mask_k) @ x[n] + b[k], y [800000, 256].

Strategy (8 cores, data-parallel over graphs, weights replicated):
  - Each core gets 100000 consecutive nodes (12500 graphs), padded to
    100352 = 49 super-tiles of 2048 nodes (256 graphs).
  - x is cast to fp16 (round-to-nearest; the PE multiplies fp16 at FP22 so
    ~2.4e-4 per-element rel err) and laid out on the host in transposed
    slice form: x_in[s, d, 128*j + n] = x[2048*s + 16*n + j, d].  Each
    slice j of a super-tile is 128 nodes, ALL of type j%8, with the
    contraction dim d already on partitions — so a contiguous 512 KiB DMA
    per super-tile feeds matmuls directly, no on-device transpose.
  - For types with dim < 128 the host writes 1.0 into x column `dim`
    (masked region), so rows 0..dim of a slice are [x.T; ones] and the
    bias rides as contraction row `dim` of the weight tile
    (y = [x,1] @ [W^T; b]).  For the two dim-128 types the (exact fp32)
    bias is added by GpSimd after eviction.
  - fp16 matmul accumulates in fp32 PSUM; pairs of slices share one PSUM
    bank [128, 512] and ScalarE/VectorE alternate evicting two slices per
    op into the fp32 out tile [128, 4096], which maps linearly to 2048
    output rows -> one contiguous 2 MiB DMA out.  All DMAs are fully
    contiguous.
W is pre-masked + pre-transposed on host (it is tiny: 1 MB).
"""

import os
import sys

import numpy as np

for _p in ("/root/.axon_site", "/root/.axon_site/_ro/trn_rl_repo", "/root/.axon_site/_ro/pypackages"):
    if os.path.isdir(_p) and _p not in sys.path:
        sys.path.append(_p)

import concourse.bass as bass
import concourse.mybir as mybir
import concourse.tile as tile
from concourse import bacc
from concourse.bass_utils import run_bass_kernel_spmd

N_TYPES = 8
MAX_DIM = 128
FEAT = 256
N_GRAPHS = 100000
NODE_DIMS = np.array([16, 32, 64, 128, 64, 32, 16, 128], dtype=np.int32)

N_CORES = 8
NODES_PER_CORE = N_GRAPHS * N_TYPES // N_CORES  # 100000
SUPER_NODES = 2048          # nodes per super-tile (256 graphs)
N_SUPER = 49                # super-tiles per core
PAD_NODES = SUPER_NODES * N_SUPER  # 100352
SLICES = SUPER_NODES // 128  # 16 slices of 128 nodes per super-tile
UNIT = 7                    # super-tiles per DMA unit (49 = 7 units of 7)
N_UNITS = N_SUPER // UNIT

_F32 = mybir.dt.float32
_F16 = mybir.dt.float16
OUT_F16 = True              # store y as fp16 (halves write traffic; host upcasts)

# PE row-strip packing: each type's contraction rows live at STRIP[k] so pairs
# of matmuls with disjoint row-groups run concurrently in the PE array:
#   (t2@0, t4@64) 64+64, (t1@0, t5@64) 33 rounds to 64, (t0@0, t6@32) 17->32,
#   t3 and t7 use the full 128 rows.
# KK[k] = contraction rows; types 0,1,5,6 append a ones-row (bias folded into
# the weight tile); types 2,4 have dim 64 (65 would round to a full-array
# tile) and types 3,7 have dim 128 — their bias is added during eviction.
STRIP = {0: 0, 1: 0, 2: 0, 3: 0, 4: 64, 5: 64, 6: 32, 7: 0}
KK = {0: 17, 1: 33, 2: 64, 3: 128, 4: 64, 5: 33, 6: 17, 7: 128}
MM_ORDER = [2, 4, 1, 5, 0, 6, 3, 7]  # pack members adjacent on the PE queue
_nc_cache = {}


def _build_nc():
    if "nc" in _nc_cache:
        return _nc_cache["nc"]
    out_dt = _F16 if OUT_F16 else _F32
    nc = bacc.Bacc("TRN2", target_bir_lowering=False, debug=False)
    x = nc.dram_tensor("x", [N_UNITS, 128, UNIT * SUPER_NODES], _F16, kind="ExternalInput").ap()
    wtb = nc.dram_tensor("wtb", [128, N_TYPES * FEAT], _F16, kind="ExternalInput").ap()
    # per-pair bias tiles for slice pairs containing an unfolded-bias type:
    # [0:512]=(2,3)->[b2|b3], [512:1024]=(4,5)->[b4|0], [1024:1536]=(6,7)->[0|b7]
    bias_pair = nc.dram_tensor("bias_pair", [128, 3 * 2 * FEAT], _F32, kind="ExternalInput").ap()
    y = nc.dram_tensor("y", [N_UNITS, 128, UNIT * SLICES * FEAT], out_dt, kind="ExternalOutput").ap()

    with tile.TileContext(nc) as tc:
        with (
            tc.tile_pool(name="const", bufs=1) as const,
            tc.tile_pool(name="xin", bufs=2) as xin_pool,
            tc.tile_pool(name="outsb", bufs=2) as out_pool,
            tc.tile_pool(name="ps_o", bufs=6, space="PSUM") as ps_o,
        ):
            wtb_sb = const.tile([128, N_TYPES * FEAT], _F16)
            nc.sync.dma_start(wtb_sb[:], wtb[:])
            bp_sb = const.tile([128, 3 * 2 * FEAT], _F32)
            nc.sync.dma_start(bp_sb[:], bias_pair[:])

            for u in range(N_UNITS):
                xs = xin_pool.tile([128, UNIT * SUPER_NODES], _F16)
                nc.sync.dma_start(xs[:], x[u])
                out_sb = out_pool.tile([128, UNIT * SLICES * FEAT], out_dt)
                for st in range(UNIT):
                    xoff = st * SUPER_NODES
                    ooff = st * SLICES * FEAT
                    for g in range(2):  # two 8-slice type-groups per super-tile
                        pos = [
                            ps_o.tile([128, 2 * FEAT], _F32, tag="po", name=f"po_{u}_{st}_{g}_{i}")
                            for i in range(4)
                        ]
                        for kt in MM_ORDER:
                            j = g * N_TYPES + kt
                            kk, sp = KK[kt], STRIP[kt]
                            nc.tensor.matmul(
                                pos[kt // 2][:, (kt % 2) * FEAT:(kt % 2 + 1) * FEAT],
                                xs[sp:sp + kk, xoff + j * 128:xoff + (j + 1) * 128],
                                wtb_sb[sp:sp + kk, kt * FEAT:(kt + 1) * FEAT],
                                start=True, stop=True,
                            )
                        for jp in range(4):
                            j0 = g * N_TYPES + 2 * jp
                            osl = out_sb[:, ooff + j0 * FEAT:ooff + (j0 + 2) * FEAT]
                            if jp == 0:   # pair (0,1): both biases folded
                                nc.scalar.copy(osl, pos[0][:])
                            else:         # pairs (2,3),(4,5),(6,7): bias in eviction
                                nc.vector.tensor_add(
                                    osl, pos[jp][:],
                                    bp_sb[:, (jp - 1) * 2 * FEAT:jp * 2 * FEAT],
                                )
                nc.scalar.dma_start(y[u], out_sb[:])

    nc.finalize()
    _nc_cache["nc"] = nc
    return nc


def _prep_weights(W, b):
    mask = (np.arange(MAX_DIM)[None, None, :] < NODE_DIMS[:, None, None])
    W_eff = np.where(mask, W, 0).astype(np.float32)  # [T, F, D]
    # wtb[:, k*256+f]: W_eff[k].T at rows STRIP[k]..STRIP[k]+dim_k, then (for
    # types with a folded bias) b[k] at row STRIP[k]+dim_k.
    wtb = np.zeros((MAX_DIM, N_TYPES * FEAT), dtype=np.float32)
    for k in range(N_TYPES):
        dim, sp, kk = int(NODE_DIMS[k]), STRIP[k], KK[k]
        wtb[sp:sp + dim, k * FEAT:(k + 1) * FEAT] = W_eff[k, :, :dim].T
        if kk == dim + 1:
            wtb[sp + dim, k * FEAT:(k + 1) * FEAT] = b[k]
    # bias_pair [128, 1536] f32: [b2|b3], [b4|0], [0|b7] broadcast over partitions
    bp = np.zeros((3, 2 * FEAT), dtype=np.float32)
    bp[0, :FEAT] = b[2]
    bp[0, FEAT:] = b[3]
    bp[1, :FEAT] = b[4]
    bp[2, FEAT:] = b[7]
    bias_pair = np.ascontiguousarray(
        np.broadcast_to(bp.reshape(1, 6 * FEAT), (128, 6 * FEAT))
    )
    return wtb.astype(np.float16), bias_pair


def _prep_x_shard(x, c):
    """fp16, ones-column injected, transposed slice layout with per-type row
    strips: out[s, STRIP[k] + d, 128*j + n] = xc[2048*s + 16*n + j, d] for
    d < KK[k], k = j % 8."""
    xc = np.zeros((PAD_NODES, MAX_DIM), dtype=np.float32)
    xc[:NODES_PER_CORE] = x[c * NODES_PER_CORE:(c + 1) * NODES_PER_CORE]
    for k in range(N_TYPES):
        dim = int(NODE_DIMS[k])
        if KK[k] == dim + 1:
            xc[k::N_TYPES, dim] = 1.0  # ones-row for the folded bias
    xh = xc.astype(np.float16).reshape(N_SUPER, 128, SLICES, MAX_DIM)  # [s, n, j, d]
    xt = np.ascontiguousarray(xh.transpose(0, 3, 2, 1))  # [s, d, j, n]
    xst = np.zeros_like(xt)  # [s, row, j, n] with per-type strip offsets
    for k in range(N_TYPES):
        sp, kk = STRIP[k], KK[k]
        for j in (k, k + N_TYPES):
            xst[:, sp:sp + kk, j, :] = xt[:, 0:kk, j, :]
    xst = xst.reshape(N_SUPER, 128, SUPER_NODES)
    # group into DMA units of UNIT super-tiles: [u, p, st*2048 + c] = xst[7u+st, p, c]
    xu = xst.reshape(N_UNITS, UNIT, 128, SUPER_NODES).transpose(0, 2, 1, 3)
    return np.ascontiguousarray(xu).reshape(N_UNITS, 128, UNIT * SUPER_NODES)


def run(x, W, b, trace=False):
    nc = _build_nc()
    wtb, bias_pair = _prep_weights(W, b)
    in_maps = []
    for c in range(N_CORES):
        in_maps.append({
            "x": _prep_x_shard(x, c),
            "wtb": wtb,
            "bias_pair": bias_pair,
        })
    res = run_bass_kernel_spmd(nc, in_maps, list(range(N_CORES)), trace=trace)
    y = np.empty((N_GRAPHS * N_TYPES, FEAT), dtype=np.float32)
    for c in range(N_CORES):
        yu = np.asarray(res.results[c]["y"]).reshape(N_UNITS, 128, UNIT, SLICES * FEAT)
        yc = yu.transpose(0, 2, 1, 3).reshape(PAD_NODES, FEAT)
        y[c * NODES_PER_CORE:(c + 1) * NODES_PER_CORE] = yc[:NODES_PER_CORE].astype(np.float32)
    return y, res


def kernel(**inputs):
    y, _ = run(inputs["x"], inputs["W"], inputs["b"])
    return y


if __name__ == "__main__":
    rng = np.random.default_rng(0)
    x = rng.standard_normal((N_GRAPHS * N_TYPES, MAX_DIM), dtype=np.float32)
    W = (rng.standard_normal((N_TYPES, FEAT, MAX_DIM), dtype=np.float32) * 0.05)
    b = (rng.standard_normal((N_TYPES, FEAT), dtype=np.float32) * 0.05)
    y, res = run(x, W, b)
    mask = (np.arange(MAX_DIM)[None, None, :] < NODE_DIMS[:, None, None])
    W_eff = np.where(mask, W, 0).astype(np.float32)
    idx = rng.integers(0, N_GRAPHS * N_TYPES, 256)
    exp = np.stack([W_eff[n % 8] @ x[n] + b[n % 8] for n in idx])
    act = y[idx]
    err = np.abs(act - exp).max() / (np.abs(exp).max() + 1e-30)
    print("spot-check rel err:", err)


# revision 35
# speedup vs baseline: 1.6259x; 1.0522x over previous
"""Trainium2 Bass kernel for nn_NodeEncoder (per-type Linear over interleaved node types).

Problem: x [800000, 128] f32, W [8, 256, 128], b [8, 256].
Node n has type k = n % 8; y[n] = (W[k] * mask_k) @ x[n] + b[k], y [800000, 256].

Strategy (8 cores, data-parallel over graphs, weights replicated):
  - Each core gets 100000 consecutive nodes (12500 graphs), padded to
    100352 = 49 super-tiles of 2048 nodes (256 graphs).
  - x is cast to fp16 (round-to-nearest; the PE multiplies fp16 at FP22 so
    ~2.4e-4 per-element rel err) and laid out on the host in transposed
    slice form: x_in[s, d, 128*j + n] = x[2048*s + 16*n + j, d].  Each
    slice j of a super-tile is 128 nodes, ALL of type j%8, with the
    contraction dim d already on partitions — so a contiguous 512 KiB DMA
    per super-tile feeds matmuls directly, no on-device transpose.
  - For types with dim < 128 the host writes 1.0 into x column `dim`
    (masked region), so rows 0..dim of a slice are [x.T; ones] and the
    bias rides as contraction row `dim` of the weight tile
    (y = [x,1] @ [W^T; b]).  For the two dim-128 types the (exact fp32)
    bias is added by GpSimd after eviction.
  - fp16 matmul accumulates in fp32 PSUM; pairs of slices share one PSUM
    bank [128, 512] and ScalarE/VectorE alternate evicting two slices per
    op into the fp32 out tile [128, 4096], which maps linearly to 2048
    output rows -> one contiguous 2 MiB DMA out.  All DMAs are fully
    contiguous.
W is pre-masked + pre-transposed on host (it is tiny: 1 MB).
"""

import os
import sys

import numpy as np

for _p in ("/root/.axon_site", "/root/.axon_site/_ro/trn_rl_repo", "/root/.axon_site/_ro/pypackages"):
    if os.path.isdir(_p) and _p not in sys.path:
        sys.path.append(_p)

import concourse.bass as bass
import concourse.mybir as mybir
import concourse.tile as tile
from concourse import bacc
from concourse.bass_utils import run_bass_kernel_spmd

N_TYPES = 8
MAX_DIM = 128
FEAT = 256
N_GRAPHS = 100000
NODE_DIMS = np.array([16, 32, 64, 128, 64, 32, 16, 128], dtype=np.int32)

N_CORES = 8
NODES_PER_CORE = N_GRAPHS * N_TYPES // N_CORES  # 100000
SUPER_NODES = 2048          # nodes per super-tile (256 graphs)
N_SUPER = 49                # super-tiles per core
PAD_NODES = SUPER_NODES * N_SUPER  # 100352
SLICES = SUPER_NODES // 128  # 16 slices of 128 nodes per super-tile
UNIT = 7                    # super-tiles per DMA unit (49 = 7 units of 7)
N_UNITS = N_SUPER // UNIT

_F32 = mybir.dt.float32
_F16 = mybir.dt.float16
OUT_F16 = True              # store y as fp16 (halves write traffic; host upcasts)

# PE row-strip packing: each type's contraction rows live at STRIP[k] so pairs
# of matmuls with disjoint row-groups run concurrently in the PE array:
#   (t2@0, t4@64) 64+64, (t1@0, t5@64) 33 rounds to 64, (t0@0, t6@32) 17->32,
#   t3 and t7 use the full 128 rows.
# KK[k] = contraction rows; types 0,1,5,6 append a ones-row (bias folded into
# the weight tile); types 2,4 have dim 64 (65 would round to a full-array
# tile) and types 3,7 have dim 128 — their bias is added during eviction.
STRIP = {0: 0, 1: 0, 2: 0, 3: 0, 4: 64, 5: 64, 6: 32, 7: 0}
KK = {0: 17, 1: 33, 2: 64, 3: 128, 4: 64, 5: 33, 6: 17, 7: 128}
MM_ORDER = [2, 4, 1, 5, 0, 6, 3, 7]  # pack members adjacent on the PE queue
_nc_cache = {}


def _build_nc():
    if "nc" in _nc_cache:
        return _nc_cache["nc"]
    out_dt = _F16 if OUT_F16 else _F32
    nc = bacc.Bacc("TRN2", target_bir_lowering=False, debug=False)
    x = nc.dram_tensor("x", [N_UNITS, 128, UNIT * SUPER_NODES], _F16, kind="ExternalInput").ap()
    wtb = nc.dram_tensor("wtb", [128, N_TYPES * FEAT], _F16, kind="ExternalInput").ap()
    # per-pair bias tiles for slice pairs containing an unfolded-bias type:
    # [0:512]=(2,3)->[b2|b3], [512:1024]=(4,5)->[b4|0], [1024:1536]=(6,7)->[0|b7]
    # (the last in fp16 for the GpSimd post-eviction add)
    bias_pair = nc.dram_tensor("bias_pair", [128, 2 * 2 * FEAT], _F32, kind="ExternalInput").ap()
    bias67 = nc.dram_tensor("bias67", [128, 2 * FEAT], _F16, kind="ExternalInput").ap()
    y = nc.dram_tensor("y", [N_UNITS, 128, UNIT * SLICES * FEAT], out_dt, kind="ExternalOutput").ap()

    with tile.TileContext(nc) as tc:
        with (
            tc.tile_pool(name="const", bufs=1) as const,
            tc.tile_pool(name="xin", bufs=2) as xin_pool,
            tc.tile_pool(name="outsb", bufs=2) as out_pool,
            tc.tile_pool(name="ps_o", bufs=6, space="PSUM") as ps_o,
        ):
            wtb_sb = const.tile([128, N_TYPES * FEAT], _F16)
            nc.sync.dma_start(wtb_sb[:], wtb[:])
            bp_sb = const.tile([128, 2 * 2 * FEAT], _F32)
            nc.sync.dma_start(bp_sb[:], bias_pair[:])
            b67_sb = const.tile([128, 2 * FEAT], _F16)
            nc.sync.dma_start(b67_sb[:], bias67[:])

            for u in range(N_UNITS):
                xs = xin_pool.tile([128, UNIT * SUPER_NODES], _F16)
                if u == 0:
                    # split the first load so compute ramps after 512 KB
                    for st in range(UNIT):
                        nc.sync.dma_start(
                            xs[:, st * SUPER_NODES:(st + 1) * SUPER_NODES],
                            x[u][:, st * SUPER_NODES:(st + 1) * SUPER_NODES],
                        )
                else:
                    nc.sync.dma_start(xs[:], x[u])
                out_sb = out_pool.tile([128, UNIT * SLICES * FEAT], out_dt)
                for st in range(UNIT):
                    xoff = st * SUPER_NODES
                    ooff = st * SLICES * FEAT
                    for g in range(2):  # two 8-slice type-groups per super-tile
                        pos = [
                            ps_o.tile([128, 2 * FEAT], _F32, tag="po", name=f"po_{u}_{st}_{g}_{i}")
                            for i in range(4)
                        ]
                        for kt in MM_ORDER:
                            j = g * N_TYPES + kt
                            kk, sp = KK[kt], STRIP[kt]
                            nc.tensor.matmul(
                                pos[kt // 2][:, (kt % 2) * FEAT:(kt % 2 + 1) * FEAT],
                                xs[sp:sp + kk, xoff + j * 128:xoff + (j + 1) * 128],
                                wtb_sb[sp:sp + kk, kt * FEAT:(kt + 1) * FEAT],
                                start=True, stop=True,
                            )
                        for jp in range(4):
                            j0 = g * N_TYPES + 2 * jp
                            osl = out_sb[:, ooff + j0 * FEAT:ooff + (j0 + 2) * FEAT]
                            if jp == 0:   # pair (0,1): both biases folded
                                nc.scalar.copy(osl, pos[0][:])
                            elif jp == 3:  # pair (6,7): ACT evict + GpSimd adds b7
                                nc.scalar.copy(osl, pos[3][:])
                                nc.gpsimd.tensor_add(osl, osl, b67_sb[:])
                            else:          # pairs (2,3),(4,5): bias in DVE eviction
                                nc.vector.tensor_add(
                                    osl, pos[jp][:],
                                    bp_sb[:, (jp - 1) * 2 * FEAT:jp * 2 * FEAT],
                                )
                # split the final store so the tail drains incrementally
                if u == N_UNITS - 1:
                    for st in range(UNIT):
                        nc.scalar.dma_start(
                            y[u][:, st * SLICES * FEAT:(st + 1) * SLICES * FEAT],
                            out_sb[:, st * SLICES * FEAT:(st + 1) * SLICES * FEAT],
                        )
                else:
                    nc.scalar.dma_start(y[u], out_sb[:])

    nc.finalize()
    _nc_cache["nc"] = nc
    return nc


def _prep_weights(W, b):
    mask = (np.arange(MAX_DIM)[None, None, :] < NODE_DIMS[:, None, None])
    W_eff = np.where(mask, W, 0).astype(np.float32)  # [T, F, D]
    # wtb[:, k*256+f]: W_eff[k].T at rows STRIP[k]..STRIP[k]+dim_k, then (for
    # types with a folded bias) b[k] at row STRIP[k]+dim_k.
    wtb = np.zeros((MAX_DIM, N_TYPES * FEAT), dtype=np.float32)
    for k in range(N_TYPES):
        dim, sp, kk = int(NODE_DIMS[k]), STRIP[k], KK[k]
        wtb[sp:sp + dim, k * FEAT:(k + 1) * FEAT] = W_eff[k, :, :dim].T
        if kk == dim + 1:
            wtb[sp + dim, k * FEAT:(k + 1) * FEAT] = b[k]
    # bias_pair [128, 1024] f32: [b2|b3], [b4|0]; bias67 [128, 512] f16: [0|b7]
    bp = np.zeros((2, 2 * FEAT), dtype=np.float32)
    bp[0, :FEAT] = b[2]
    bp[0, FEAT:] = b[3]
    bp[1, :FEAT] = b[4]
    bias_pair = np.ascontiguousarray(
        np.broadcast_to(bp.reshape(1, 4 * FEAT), (128, 4 * FEAT))
    )
    b67 = np.zeros((1, 2 * FEAT), dtype=np.float16)
    b67[0, FEAT:] = b[7].astype(np.float16)
    bias67 = np.ascontiguousarray(np.broadcast_to(b67, (128, 2 * FEAT)))
    return wtb.astype(np.float16), bias_pair, bias67


def _prep_x_shard(x, c):
    """fp16, ones-column injected, transposed slice layout with per-type row
    strips: out[s, STRIP[k] + d, 128*j + n] = xc[2048*s + 16*n + j, d] for
    d < KK[k], k = j % 8."""
    xc = np.zeros((PAD_NODES, MAX_DIM), dtype=np.float32)
    xc[:NODES_PER_CORE] = x[c * NODES_PER_CORE:(c + 1) * NODES_PER_CORE]
    for k in range(N_TYPES):
        dim = int(NODE_DIMS[k])
        if KK[k] == dim + 1:
            xc[k::N_TYPES, dim] = 1.0  # ones-row for the folded bias
    xh = xc.astype(np.float16).reshape(N_SUPER, 128, SLICES, MAX_DIM)  # [s, n, j, d]
    xt = np.ascontiguousarray(xh.transpose(0, 3, 2, 1))  # [s, d, j, n]
    xst = np.zeros_like(xt)  # [s, row, j, n] with per-type strip offsets
    for k in range(N_TYPES):
        sp, kk = STRIP[k], KK[k]
        for j in (k, k + N_TYPES):
            xst[:, sp:sp + kk, j, :] = xt[:, 0:kk, j, :]
    xst = xst.reshape(N_SUPER, 128, SUPER_NODES)
    # group into DMA units of UNIT super-tiles: [u, p, st*2048 + c] = xst[7u+st, p, c]
    xu = xst.reshape(N_UNITS, UNIT, 128, SUPER_NODES).transpose(0, 2, 1, 3)
    return np.ascontiguousarray(xu).reshape(N_UNITS, 128, UNIT * SUPER_NODES)


def run(x, W, b, trace=False):
    nc = _build_nc()
    wtb, bias_pair, bias67 = _prep_weights(W, b)
    in_maps = []
    for c in range(N_CORES):
        in_maps.append({
            "x": _prep_x_shard(x, c),
            "wtb": wtb,
            "bias_pair": bias_pair,
            "bias67": bias67,
        })
    res = run_bass_kernel_spmd(nc, in_maps, list(range(N_CORES)), trace=trace)
    y = np.empty((N_GRAPHS * N_TYPES, FEAT), dtype=np.float32)
    for c in range(N_CORES):
        yu = np.asarray(res.results[c]["y"]).reshape(N_UNITS, 128, UNIT, SLICES * FEAT)
        yc = yu.transpose(0, 2, 1, 3).reshape(PAD_NODES, FEAT)
        y[c * NODES_PER_CORE:(c + 1) * NODES_PER_CORE] = yc[:NODES_PER_CORE].astype(np.float32)
    return y, res


def kernel(**inputs):
    y, _ = run(inputs["x"], inputs["W"], inputs["b"])
    return y


if __name__ == "__main__":
    rng = np.random.default_rng(0)
    x = rng.standard_normal((N_GRAPHS * N_TYPES, MAX_DIM), dtype=np.float32)
    W = (rng.standard_normal((N_TYPES, FEAT, MAX_DIM), dtype=np.float32) * 0.05)
    b = (rng.standard_normal((N_TYPES, FEAT), dtype=np.float32) * 0.05)
    y, res = run(x, W, b)
    mask = (np.arange(MAX_DIM)[None, None, :] < NODE_DIMS[:, None, None])
    W_eff = np.where(mask, W, 0).astype(np.float32)
    idx = rng.integers(0, N_GRAPHS * N_TYPES, 256)
    exp = np.stack([W_eff[n % 8] @ x[n] + b[n % 8] for n in idx])
    act = y[idx]
    err = np.abs(act - exp).max() / (np.abs(exp).max() + 1e-30)
    print("spot-check rel err:", err)


# revision 40
# speedup vs baseline: 1.6674x; 1.0256x over previous
"""Trainium2 Bass kernel for nn_NodeEncoder (per-type Linear over interleaved node types).

Problem: x [800000, 128] f32, W [8, 256, 128], b [8, 256].
Node n has type k = n % 8; y[n] = (W[k] * mask_k) @ x[n] + b[k], y [800000, 256].

Strategy (8 cores, data-parallel over graphs, weights replicated):
  - Each core gets 100000 consecutive nodes (12500 graphs), padded to
    100352 = 49 super-tiles of 2048 nodes (256 graphs).
  - x is cast to fp16 (round-to-nearest; the PE multiplies fp16 at FP22 so
    ~2.4e-4 per-element rel err) and laid out on the host in transposed
    slice form: x_in[s, d, 128*j + n] = x[2048*s + 16*n + j, d].  Each
    slice j of a super-tile is 128 nodes, ALL of type j%8, with the
    contraction dim d already on partitions — so a contiguous 512 KiB DMA
    per super-tile feeds matmuls directly, no on-device transpose.
  - For types with dim < 128 the host writes 1.0 into x column `dim`
    (masked region), so rows 0..dim of a slice are [x.T; ones] and the
    bias rides as contraction row `dim` of the weight tile
    (y = [x,1] @ [W^T; b]).  For the two dim-128 types the (exact fp32)
    bias is added by GpSimd after eviction.
  - fp16 matmul accumulates in fp32 PSUM; pairs of slices share one PSUM
    bank [128, 512] and ScalarE/VectorE alternate evicting two slices per
    op into the fp32 out tile [128, 4096], which maps linearly to 2048
    output rows -> one contiguous 2 MiB DMA out.  All DMAs are fully
    contiguous.
W is pre-masked + pre-transposed on host (it is tiny: 1 MB).
"""

import os
import sys

import numpy as np

for _p in ("/root/.axon_site", "/root/.axon_site/_ro/trn_rl_repo", "/root/.axon_site/_ro/pypackages"):
    if os.path.isdir(_p) and _p not in sys.path:
        sys.path.append(_p)

import concourse.bass as bass
import concourse.mybir as mybir
import concourse.tile as tile
from concourse import bacc
from concourse.bass_utils import run_bass_kernel_spmd

N_TYPES = 8
MAX_DIM = 128
FEAT = 256
N_GRAPHS = 100000
NODE_DIMS = np.array([16, 32, 64, 128, 64, 32, 16, 128], dtype=np.int32)

N_CORES = 8
NODES_PER_CORE = N_GRAPHS * N_TYPES // N_CORES  # 100000
SUPER_NODES = 2048          # nodes per super-tile (256 graphs)
N_SUPER = 49                # super-tiles per core
PAD_NODES = SUPER_NODES * N_SUPER  # 100352
SLICES = SUPER_NODES // 128  # 16 slices of 128 nodes per super-tile
UNIT = 7                    # super-tiles per DMA unit (49 = 7 units of 7)
N_UNITS = N_SUPER // UNIT

_F32 = mybir.dt.float32
_F16 = mybir.dt.float16
OUT_F16 = True              # store y as fp16 (halves write traffic; host upcasts)

# PE row-strip packing: each type's contraction rows live at STRIP[k] so pairs
# of matmuls with disjoint row-groups run concurrently in the PE array:
#   (t2@0, t4@64) 64+64, (t1@0, t5@64) 33 rounds to 64, (t0@0, t6@32) 17->32,
#   t3 and t7 use the full 128 rows.
# KK[k] = contraction rows; types 0,1,5,6 append a ones-row (bias folded into
# the weight tile); types 2,4 have dim 64 (65 would round to a full-array
# tile) and types 3,7 have dim 128 — their bias is added during eviction.
STRIP = {0: 0, 1: 0, 2: 0, 3: 0, 4: 64, 5: 64, 6: 32, 7: 0}
KK = {0: 17, 1: 33, 2: 64, 3: 128, 4: 64, 5: 33, 6: 17, 7: 128}
MM_ORDER = [2, 4, 1, 5, 0, 6, 3, 7]  # pack members adjacent on the PE queue
_nc_cache = {}


def _build_nc():
    if "nc" in _nc_cache:
        return _nc_cache["nc"]
    out_dt = _F16 if OUT_F16 else _F32
    nc = bacc.Bacc("TRN2", target_bir_lowering=False, debug=False)
    x = nc.dram_tensor("x", [N_UNITS, 128, UNIT * SUPER_NODES], _F16, kind="ExternalInput").ap()
    wtb = nc.dram_tensor("wtb", [128, N_TYPES * FEAT], _F16, kind="ExternalInput").ap()
    # bias tiles for the unfolded-bias types, broadcast over partitions:
    # [0:512] = [b2|b3] (pair eviction), [512:768] = b4, [768:1024] = b7
    bias_pair = nc.dram_tensor("bias_pair", [128, 4 * FEAT], _F32, kind="ExternalInput").ap()
    y = nc.dram_tensor("y", [N_UNITS, 128, UNIT * SLICES * FEAT], out_dt, kind="ExternalOutput").ap()

    with tile.TileContext(nc) as tc:
        with (
            tc.tile_pool(name="const", bufs=1) as const,
            tc.tile_pool(name="xin", bufs=2) as xin_pool,
            tc.tile_pool(name="outsb", bufs=2) as out_pool,
            tc.tile_pool(name="ps_o", bufs=6, space="PSUM") as ps_o,
        ):
            wtb_sb = const.tile([128, N_TYPES * FEAT], _F16)
            nc.sync.dma_start(wtb_sb[:], wtb[:])
            bp_sb = const.tile([128, 4 * FEAT], _F32)
            nc.sync.dma_start(bp_sb[:], bias_pair[:])

            for u in range(N_UNITS):
                xs = xin_pool.tile([128, UNIT * SUPER_NODES], _F16)
                if u == 0:
                    # split the first load so compute ramps after 512 KB
                    for st in range(UNIT):
                        nc.sync.dma_start(
                            xs[:, st * SUPER_NODES:(st + 1) * SUPER_NODES],
                            x[u][:, st * SUPER_NODES:(st + 1) * SUPER_NODES],
                        )
                else:
                    nc.sync.dma_start(xs[:], x[u])
                out_sb = out_pool.tile([128, UNIT * SLICES * FEAT], out_dt)
                for st in range(UNIT):
                    xoff = st * SUPER_NODES
                    ooff = st * SLICES * FEAT
                    for g in range(2):  # two 8-slice type-groups per super-tile
                        pos = [
                            ps_o.tile([128, 2 * FEAT], _F32, tag="po", name=f"po_{u}_{st}_{g}_{i}")
                            for i in range(4)
                        ]
                        for kt in MM_ORDER:
                            j = g * N_TYPES + kt
                            kk, sp = KK[kt], STRIP[kt]
                            nc.tensor.matmul(
                                pos[kt // 2][:, (kt % 2) * FEAT:(kt % 2 + 1) * FEAT],
                                xs[sp:sp + kk, xoff + j * 128:xoff + (j + 1) * 128],
                                wtb_sb[sp:sp + kk, kt * FEAT:(kt + 1) * FEAT],
                                start=True, stop=True,
                            )
                        # evictions: biased halves on DVE tensor_add (bias folded
                        # into the PSUM->SBUF move, single fp16 rounding),
                        # unbiased halves on ScalarE copy.
                        jb = g * N_TYPES
                        oss = [
                            out_sb[:, ooff + (jb + i) * FEAT:ooff + (jb + i + 1) * FEAT]
                            for i in range(N_TYPES)
                        ]
                        nc.scalar.copy(out_sb[:, ooff + jb * FEAT:ooff + (jb + 2) * FEAT], pos[0][:])
                        nc.vector.tensor_add(
                            out_sb[:, ooff + (jb + 2) * FEAT:ooff + (jb + 4) * FEAT],
                            pos[1][:], bp_sb[:, 0:2 * FEAT],
                        )
                        nc.vector.tensor_add(oss[4], pos[2][:, 0:FEAT], bp_sb[:, 2 * FEAT:3 * FEAT])
                        nc.scalar.copy(oss[5], pos[2][:, FEAT:2 * FEAT])
                        nc.scalar.copy(oss[6], pos[3][:, 0:FEAT])
                        nc.vector.tensor_add(oss[7], pos[3][:, FEAT:2 * FEAT], bp_sb[:, 3 * FEAT:4 * FEAT])
                # split the final store so the tail drains incrementally
                if u == N_UNITS - 1:
                    for st in range(UNIT):
                        nc.scalar.dma_start(
                            y[u][:, st * SLICES * FEAT:(st + 1) * SLICES * FEAT],
                            out_sb[:, st * SLICES * FEAT:(st + 1) * SLICES * FEAT],
                        )
                else:
                    nc.scalar.dma_start(y[u], out_sb[:])

    nc.finalize()
    _nc_cache["nc"] = nc
    return nc


def _prep_weights(W, b):
    mask = (np.arange(MAX_DIM)[None, None, :] < NODE_DIMS[:, None, None])
    W_eff = np.where(mask, W, 0).astype(np.float32)  # [T, F, D]
    # wtb[:, k*256+f]: W_eff[k].T at rows STRIP[k]..STRIP[k]+dim_k, then (for
    # types with a folded bias) b[k] at row STRIP[k]+dim_k.
    wtb = np.zeros((MAX_DIM, N_TYPES * FEAT), dtype=np.float32)
    for k in range(N_TYPES):
        dim, sp, kk = int(NODE_DIMS[k]), STRIP[k], KK[k]
        wtb[sp:sp + dim, k * FEAT:(k + 1) * FEAT] = W_eff[k, :, :dim].T
        if kk == dim + 1:
            wtb[sp + dim, k * FEAT:(k + 1) * FEAT] = b[k]
    # bias_pair [128, 1024] f32: [b2 | b3 | b4 | b7] broadcast over partitions
    bp = np.concatenate([b[2], b[3], b[4], b[7]]).astype(np.float32)[None, :]
    bias_pair = np.ascontiguousarray(np.broadcast_to(bp, (128, 4 * FEAT)))
    return wtb.astype(np.float16), bias_pair


def _prep_x_shard(x, c):
    """fp16, ones-column injected, transposed slice layout with per-type row
    strips: out[s, STRIP[k] + d, 128*j + n] = xc[2048*s + 16*n + j, d] for
    d < KK[k], k = j % 8."""
    xc = np.zeros((PAD_NODES, MAX_DIM), dtype=np.float32)
    xc[:NODES_PER_CORE] = x[c * NODES_PER_CORE:(c + 1) * NODES_PER_CORE]
    for k in range(N_TYPES):
        dim = int(NODE_DIMS[k])
        if KK[k] == dim + 1:
            xc[k::N_TYPES, dim] = 1.0  # ones-row for the folded bias
    xh = xc.astype(np.float16).reshape(N_SUPER, 128, SLICES, MAX_DIM)  # [s, n, j, d]
    xt = np.ascontiguousarray(xh.transpose(0, 3, 2, 1))  # [s, d, j, n]
    xst = np.zeros_like(xt)  # [s, row, j, n] with per-type strip offsets
    for k in range(N_TYPES):
        sp, kk = STRIP[k], KK[k]
        for j in (k, k + N_TYPES):
            xst[:, sp:sp + kk, j, :] = xt[:, 0:kk, j, :]
    xst = xst.reshape(N_SUPER, 128, SUPER_NODES)
    # group into DMA units of UNIT super-tiles: [u, p, st*2048 + c] = xst[7u+st, p, c]
    xu = xst.reshape(N_UNITS, UNIT, 128, SUPER_NODES).transpose(0, 2, 1, 3)
    return np.ascontiguousarray(xu).reshape(N_UNITS, 128, UNIT * SUPER_NODES)


def run(x, W, b, trace=False):
    nc = _build_nc()
    wtb, bias_pair = _prep_weights(W, b)
    in_maps = []
    for c in range(N_CORES):
        in_maps.append({
            "x": _prep_x_shard(x, c),
            "wtb": wtb,
            "bias_pair": bias_pair,
        })
    res = run_bass_kernel_spmd(nc, in_maps, list(range(N_CORES)), trace=trace)
    y = np.empty((N_GRAPHS * N_TYPES, FEAT), dtype=np.float32)
    for c in range(N_CORES):
        yu = np.asarray(res.results[c]["y"]).reshape(N_UNITS, 128, UNIT, SLICES * FEAT)
        yc = yu.transpose(0, 2, 1, 3).reshape(PAD_NODES, FEAT)
        y[c * NODES_PER_CORE:(c + 1) * NODES_PER_CORE] = yc[:NODES_PER_CORE].astype(np.float32)
    return y, res


def kernel(**inputs):
    y, _ = run(inputs["x"], inputs["W"], inputs["b"])
    return y


if __name__ == "__main__":
    rng = np.random.default_rng(0)
    x = rng.standard_normal((N_GRAPHS * N_TYPES, MAX_DIM), dtype=np.float32)
    W = (rng.standard_normal((N_TYPES, FEAT, MAX_DIM), dtype=np.float32) * 0.05)
    b = (rng.standard_normal((N_TYPES, FEAT), dtype=np.float32) * 0.05)
    y, res = run(x, W, b)
    mask = (np.arange(MAX_DIM)[None, None, :] < NODE_DIMS[:, None, None])
    W_eff = np.where(mask, W, 0).astype(np.float32)
    idx = rng.integers(0, N_GRAPHS * N_TYPES, 256)
    exp = np.stack([W_eff[n % 8] @ x[n] + b[n % 8] for n in idx])
    act = y[idx]
    err = np.abs(act - exp).max() / (np.abs(exp).max() + 1e-30)
    print("spot-check rel err:", err)
